# revision 3
# baseline (speedup 1.0000x reference)
"""Trainium2 Bass kernel for nn_KernelProjectionT2I — v2.

Sharding: data-parallel over captions (B_cap=48 -> 6 per core on 8 cores).
Each core holds the full image batch + conv weights and computes its 6
similarity columns; the host concatenates.

Host precompute (cheap, O(Q*D) math):
  caption MLP + tap softmax -> per-caption tap weights w0/w1/w2 (Q, D)
  cap l2-normalization; final sims = dot / sqrt(s2) division.

Device math per caption q (layout: channels c on partitions for the tap
chain, image-region pixels n=(b,r) on PSUM partitions for y):
  xcv = bf16(w0*x[r-1] + w1*x[r] + w2*x[r+1])     tap chain (DVE)
  y   = Wconv @ xcv                                bf16 matmuls, f32 PSUM
  e   = bf16(exp(y)); yb = bf16(y)                 ACT (frees PSUM fast)
  p   = bf16(yb * e)                               DVE 2x (all-bf16 SBUF)
  A   = sum_r e ; Bs = sum_r p                     0/1-selector bf16 matmuls
                                                   (caption pair packed into
                                                   64+64 output partitions)
  v   = Bs/A + bconv                               f32 epilogue (DVE)
  dot = <v, capn>; s2 = <v, v>                     fused TT-reduce

Outputs per core: dot/s2 [128, 3+3] f32; host finishes sims = dot/sqrt(s2).
"""

import os
import numpy as np
import ml_dtypes
from contextlib import ExitStack

import concourse.bass as bass
import concourse.tile as tile
from concourse import bacc, mybir
from concourse.bass_utils import run_bass_kernel_spmd

F32 = mybir.dt.float32
BF16 = mybir.dt.bfloat16
AF = mybir.ActivationFunctionType
OP = mybir.AluOpType

NPBF = ml_dtypes.bfloat16

N_CORES = 8
B, R, D = 48, 36, 1024
Q = 48
QL = Q // N_CORES            # 6 captions per core
NPAIR = QL // 2              # 3 caption pairs
NB = B * R                   # 1728
NP = 1792                    # padded n, 14 chunks of 128
NCH = NP // 128              # 14

# cc's whose tap2 (t0) multiply runs on ACT (rest DVE tensor_scalar)
T0_ACT = frozenset()

LAST_EXEC_NS = None
_CACHE = {}


def _build_nc():
    nc = bacc.Bacc(trn_type="TRN2", target_bir_lowering=False,
                   num_devices=N_CORES)
    x38_d = nc.dram_tensor("x38", [128, 8, B, 38], BF16, kind="ExternalInput")
    wct_d = nc.dram_tensor("wct", [128, 8, D], BF16, kind="ExternalInput")
    sel_d = nc.dram_tensor("sel", [128, 2, NCH, 128], BF16,
                           kind="ExternalInput")
    w012_d = nc.dram_tensor("w012", [128, 8, 3, QL], F32,
                            kind="ExternalInput")
    capn_d = nc.dram_tensor("capn", [128, NPAIR, D], F32,
                            kind="ExternalInput")
    bcb_d = nc.dram_tensor("bcb", [128, D], F32, kind="ExternalInput")
    out_d = nc.dram_tensor("out", [128, 2 * NPAIR], F32, kind="ExternalOutput")

    with ExitStack() as ctx:
        tc = ctx.enter_context(tile.TileContext(nc))
        const = ctx.enter_context(tc.tile_pool(name="const", bufs=1))
        tap = ctx.enter_context(tc.tile_pool(name="tap", bufs=3))
        xcp = ctx.enter_context(tc.tile_pool(name="xcp", bufs=2))
        ep = ctx.enter_context(tc.tile_pool(name="ep", bufs=4))
        epi = ctx.enter_context(tc.tile_pool(name="epi", bufs=1))
        psy = ctx.enter_context(tc.tile_pool(name="psy", bufs=4, space="PSUM"))
        psA = ctx.enter_context(tc.tile_pool(name="psA", bufs=1, space="PSUM"))
        psB = ctx.enter_context(tc.tile_pool(name="psB", bufs=1, space="PSUM"))

        # ---- resident inputs (order = DMA priority) ----
        w012_t = const.tile([128, 8, 3, QL], F32)
        nc.sync.dma_start(out=w012_t, in_=w012_d.ap())
        sel_t = const.tile([128, 2, NCH, 128], BF16)
        nc.sync.dma_start(out=sel_t, in_=sel_d.ap())
        x38_t = const.tile([128, 8, B, 38], BF16)
        for bq in range(0, B, 12):
            nc.sync.dma_start(out=x38_t[:, :, bq:bq + 12, :],
                              in_=x38_d.ap()[:, :, bq:bq + 12, :])
        wct_t = const.tile([128, 8, D], BF16)
        nc.sync.dma_start(out=wct_t, in_=wct_d.ap())
        bcb_t = const.tile([128, D], F32)
        nc.sync.dma_start(out=bcb_t, in_=bcb_d.ap())
        capn_t = const.tile([128, NPAIR, D], F32)
        nc.sync.dma_start(out=capn_t, in_=capn_d.ap())
        dot_t = const.tile([128, 2 * NPAIR], F32)

        def sel_mms(S, j, eb, pb, A_ps, B_ps):
            selj = sel_t[:, S, j, :]
            for h in range(2):
                hs = slice(h * 512, (h + 1) * 512)
                nc.tensor.matmul(A_ps[:, hs], lhsT=selj, rhs=eb[:, hs],
                                 start=(S == 0 and j == 0),
                                 stop=(S == 1 and j == NCH - 1))
                nc.tensor.matmul(B_ps[:, hs], lhsT=selj, rhs=pb[:, hs],
                                 start=(S == 0 and j == 0),
                                 stop=(S == 1 and j == NCH - 1))

        # xcv tiles, allocated lazily; X-stage emitted block-by-block so
        # caption q+1's tap chain interleaves into caption q's chunk loop
        # (engine FIFOs are in-order: without interleaving, q+1's DVE/ACT
        # work queues behind q's PSUM-gated p-passes and PE stalls at every
        # caption boundary)
        xcv_of = {}

        def get_xcv(q):
            if q not in xcv_of:
                xcv_of[q] = xcp.tile([128, 8, NP], BF16, tag="xc",
                                     name=f"xcv{q}")
                nc.vector.memset(xcv_of[q][:, :, NB:NP], 0.0)
            return xcv_of[q]

        def emit_x_block(q, blk, nb=24):
            half, cc = blk >> 3, blk & 7
            xcvq = get_xcv(q)
            b0, b1 = half * nb, half * nb + nb
            xs = lambda o: x38_t[:, cc, b0:b1, o:o + 36]
            t0 = tap.tile([128, 24, 36], BF16, tag="t0", name="t0")[:, 0:nb, :]
            if cc in T0_ACT:
                nc.scalar.mul(t0, xs(2), w012_t[:, cc, 2, q:q + 1])
            else:
                nc.vector.tensor_scalar_mul(t0, xs(2),
                                            w012_t[:, cc, 2, q:q + 1])
            u0 = tap.tile([128, 24, 36], BF16, tag="u0", name="u0")[:, 0:nb, :]
            nc.vector.tensor_scalar_mul(u0, xs(1), w012_t[:, cc, 1, q:q + 1])
            u1 = tap.tile([128, 24, 36], BF16, tag="u1", name="u1")[:, 0:nb, :]
            nc.vector.tensor_tensor(u1, u0, t0, OP.add)
            xo = xcvq[:, cc, b0 * 36:b1 * 36].rearrange(
                "p (b r) -> p b r", r=36)
            nc.vector.scalar_tensor_tensor(
                xo, xs(0), w012_t[:, cc, 0, q:q + 1], u1, OP.mult, OP.add)

        # first caption in b-quarters (quarter-major) so that chunk-0
        # matmuls can start as soon as the first quarter lands
        for blk in range(32):
            emit_x_block(0, blk, nb=12)

        for P in range(NPAIR):
            A_ps = psA.tile([128, D], F32, tag="A")
            B_ps = psB.tile([128, D], F32, tag="B")
            for S in range(2):
                q = 2 * P + S
                xcv = get_xcv(q)
                nxt = 0

                # ---- M/E/S over 14 n-chunks; selector MMs lag two chunks
                # so PE never waits on the just-produced e/p tiles ----
                lagq = []
                for j in range(NCH):
                    eb = ep.tile([128, D], BF16, tag="e", name="eb")
                    pb = ep.tile([128, D], BF16, tag="p", name="pb")
                    yb = ep.tile([128, D], BF16, tag="yb", name="yb")
                    for h in range(2):
                        hs = slice(h * 512, (h + 1) * 512)
                        y_ps = psy.tile([128, 512], F32, tag="y")
                        for cc in range(8):
                            nc.tensor.matmul(
                                y_ps,
                                lhsT=xcv[:, cc, j * 128:(j + 1) * 128],
                                rhs=wct_t[:, cc, hs],
                                start=(cc == 0), stop=(cc == 7))
                        # both PSUM consumers on ACT so the y buffer frees at
                        # ACT's pace; p then runs all-bf16 on DVE at 2x
                        nc.scalar.activation(eb[:, hs], y_ps, AF.Exp)
                        nc.scalar.copy(yb[:, hs], y_ps)
                        nc.vector.tensor_tensor(pb[:, hs], yb[:, hs],
                                                eb[:, hs], OP.mult)
                    lagq.append((j, eb, pb))
                    if len(lagq) > 2:
                        jj, e2, p2 = lagq.pop(0)
                        sel_mms(S, jj, e2, p2, A_ps, B_ps)
                    # pace next caption's tap chain into this loop
                    if q + 1 < QL:
                        while nxt < (j + 1) * 16 // NCH:
                            emit_x_block(q + 1, nxt)
                            nxt += 1
                for jj, e2, p2 in lagq:
                    sel_mms(S, jj, e2, p2, A_ps, B_ps)

            # ---- epilogue per caption pair (f32) ----
            # 1/A via exp(-ln(A)) on ScalarE (A > 0): custom-DVE reciprocal
            # is unsupported on this runtime
            rA = epi.tile([128, D], F32, tag="rA")
            nc.scalar.activation(rA, A_ps, AF.Ln)
            nc.scalar.activation(rA, rA, AF.Exp, scale=-1.0)
            vb = epi.tile([128, D], F32, tag="vb")
            nc.vector.scalar_tensor_tensor(vb, B_ps, 1.0, rA,
                                           OP.mult, OP.mult)
            nc.vector.tensor_tensor(vb, vb, bcb_t, OP.add)
            scr = epi.tile([128, D], F32, tag="scr")
            nc.vector.tensor_tensor(scr, vb, capn_t[:, P, :], OP.mult)
            nc.vector.tensor_reduce(dot_t[:, P:P + 1], scr,
                                    mybir.AxisListType.X, OP.add)
            nc.vector.tensor_tensor(scr, vb, vb, OP.mult)
            nc.vector.tensor_reduce(dot_t[:, NPAIR + P:NPAIR + P + 1], scr,
                                    mybir.AxisListType.X, OP.add)

        nc.sync.dma_start(out=out_d.ap(), in_=dot_t)

    nc.compile()
    return nc


def _host_prep(img, cap0, Wred, bred, Wproj, bproj, Wconv):
    """Shared (caption-independent) tensors + per-caption tap weights."""
    capr = cap0 @ Wred.T + bred
    wdyn = (capr @ Wproj.T + bproj).reshape(Q, D, 3)
    wdyn = np.exp(wdyn - wdyn.max(-1, keepdims=True))
    wdyn /= wdyn.sum(-1, keepdims=True)                    # (Q, D, 3)

    xt = img.transpose(2, 0, 1)                            # (D, B, R)
    x38 = np.zeros((D, B, 38), np.float32)
    x38[:, :, 1:37] = xt
    x38 = np.ascontiguousarray(
        x38.reshape(8, 128, B, 38).transpose(1, 0, 2, 3)).astype(NPBF)

    # wct[p, cc, d] = Wconv[d, c], c = 128 cc + p
    wct = np.ascontiguousarray(
        Wconv.T.reshape(8, 128, D).transpose(1, 0, 2)).astype(NPBF)

    # 0/1 selectors: S=0 -> out cols 0..63, S=1 -> 64..127
    sel = np.zeros((128, 2, NCH, 128), np.float32)
    n = np.arange(NB)
    p, ch = n % 128, n // 128
    for s in range(2):
        sel[p, s, ch, 64 * s + n // R] = 1.0
        # pad out-rows (b 48..63 per slot) tied to invalid-n rows of chunk 13
        # (xcv zeros there -> e = 1, p = 0) so A accumulates 1.0 and the
        # epilogue reciprocal stays finite
        for t in range(16):
            sel[64 + t, s, NCH - 1, 64 * s + 48 + t] = 1.0
    return wdyn, x38, wct, sel.astype(NPBF)


def kernel(img_embed, cap_embed, lens, Wred, bred, Wproj, bproj, Wconv,
           bconv, **_unused):
    global LAST_EXEC_NS
    img_embed = np.asarray(img_embed, np.float32)
    cap0 = np.asarray(cap_embed, np.float32)[:, 0, :]
    Wred = np.asarray(Wred, np.float32)
    bred_a = np.asarray(bred, np.float32)
    Wproj = np.asarray(Wproj, np.float32)
    bproj_a = np.asarray(bproj, np.float32)
    Wconv = np.asarray(Wconv, np.float32)
    bconv_a = np.asarray(bconv, np.float32)

    if "nc" not in _CACHE:
        _CACHE["nc"] = _build_nc()
    nc = _CACHE["nc"]

    wdyn, x38, wct, sel = _host_prep(img_embed, cap0, Wred, bred_a,
                                     Wproj, bproj_a, Wconv)
    capn = cap0 / np.linalg.norm(cap0, axis=1, keepdims=True)   # (Q, D)
    bcb = np.ascontiguousarray(
        np.broadcast_to(bconv_a, (128, D))).astype(np.float32)

    in_maps = []
    for c in range(N_CORES):
        qs = slice(c * QL, (c + 1) * QL)
        # w012[p, cc, tap, q] = wdyn[q_local, 128 cc + p, tap]
        w012 = np.ascontiguousarray(
            wdyn[qs].reshape(QL, 8, 128, 3).transpose(2, 1, 3, 0)
        ).astype(np.float32)
        # capn pairs: partitions 0..63 even caption, 64..127 odd
        cpn = np.empty((128, NPAIR, D), np.float32)
        for P in range(NPAIR):
            cpn[0:64, P] = capn[c * QL + 2 * P]
            cpn[64:128, P] = capn[c * QL + 2 * P + 1]
        in_maps.append({
            "x38": x38, "wct": wct, "sel": sel, "w012": w012,
            "capn": cpn, "bcb": bcb,
        })

    trace = bool(int(os.environ.get("KTRACE", "0")))
    tdir = os.environ.get("KTRACE_DIR") or None
    res = run_bass_kernel_spmd(nc, in_maps, core_ids=list(range(N_CORES)),
                               trace=trace, tmpdir=tdir)
    LAST_EXEC_NS = res.exec_time_ns

    sims = np.empty((B, Q), np.float32)
    for c in range(N_CORES):
        o = np.asarray(res.results[c]["out"], np.float32)   # (128, 6)
        for P in range(NPAIR):
            for S in range(2):
                rows = slice(64 * S, 64 * S + 48)
                dot = o[rows, P]
                s2 = o[rows, NPAIR + P]
                sims[:, c * QL + 2 * P + S] = dot / np.sqrt(s2)
    return sims


# revision 4
# speedup vs baseline: 1.0468x; 1.0468x over previous
"""Trainium2 Bass kernel for nn_KernelProjectionT2I — v2.

Sharding: data-parallel over captions (B_cap=48 -> 6 per core on 8 cores).
Each core holds the full image batch + conv weights and computes its 6
similarity columns; the host concatenates.

Host precompute (cheap, O(Q*D) math):
  caption MLP + tap softmax -> per-caption tap weights w0/w1/w2 (Q, D)
  cap l2-normalization; final sims = dot / sqrt(s2) division.

Device math per caption q (layout: channels c on partitions for the tap
chain, image-region pixels n=(b,r) on PSUM partitions for y):
  xcv = bf16(w0*x[r-1] + w1*x[r] + w2*x[r+1])     tap chain (DVE)
  y   = Wconv @ xcv                                bf16 matmuls, f32 PSUM
  e   = bf16(exp(y)); yb = bf16(y)                 ACT (frees PSUM fast)
  p   = bf16(yb * e)                               DVE 2x (all-bf16 SBUF)
  A   = sum_r e ; Bs = sum_r p                     0/1-selector bf16 matmuls
                                                   (caption pair packed into
                                                   64+64 output partitions)
  v   = Bs/A + bconv                               f32 epilogue (DVE)
  dot = <v, capn>; s2 = <v, v>                     fused TT-reduce

Outputs per core: dot/s2 [128, 3+3] f32; host finishes sims = dot/sqrt(s2).
"""

import os
import numpy as np
import ml_dtypes
from contextlib import ExitStack

import concourse.bass as bass
import concourse.tile as tile
from concourse import bacc, mybir
from concourse.bass_utils import run_bass_kernel_spmd

F32 = mybir.dt.float32
BF16 = mybir.dt.bfloat16
AF = mybir.ActivationFunctionType
OP = mybir.AluOpType

NPBF = ml_dtypes.bfloat16

N_CORES = 8
B, R, D = 48, 36, 1024
Q = 48
QL = Q // N_CORES            # 6 captions per core
NPAIR = QL // 2              # 3 caption pairs
NB = B * R                   # 1728
NP = 1792                    # padded n, 14 chunks of 128
NCH = NP // 128              # 14

# cc's whose tap2 (t0) multiply runs on ACT (rest DVE tensor_scalar)
T0_ACT = frozenset()

LAST_EXEC_NS = None
_CACHE = {}


def _build_nc():
    nc = bacc.Bacc(trn_type="TRN2", target_bir_lowering=False,
                   num_devices=N_CORES)
    x38_d = nc.dram_tensor("x38", [128, 8, B, 38], BF16, kind="ExternalInput")
    wct_d = nc.dram_tensor("wct", [128, 8, D], BF16, kind="ExternalInput")
    sel_d = nc.dram_tensor("sel", [128, 2, NCH, 128], BF16,
                           kind="ExternalInput")
    w012_d = nc.dram_tensor("w012", [128, 8, 3, QL], F32,
                            kind="ExternalInput")
    capn_d = nc.dram_tensor("capn", [128, NPAIR, D], F32,
                            kind="ExternalInput")
    bcb_d = nc.dram_tensor("bcb", [128, D], F32, kind="ExternalInput")
    out_d = nc.dram_tensor("out", [128, 2 * NPAIR], F32, kind="ExternalOutput")

    with ExitStack() as ctx:
        tc = ctx.enter_context(tile.TileContext(nc))
        const = ctx.enter_context(tc.tile_pool(name="const", bufs=1))
        tap = ctx.enter_context(tc.tile_pool(name="tap", bufs=3))
        xcp = ctx.enter_context(tc.tile_pool(name="xcp", bufs=2))
        ep = ctx.enter_context(tc.tile_pool(name="ep", bufs=4))
        epi = ctx.enter_context(tc.tile_pool(name="epi", bufs=1))
        psy = ctx.enter_context(tc.tile_pool(name="psy", bufs=4, space="PSUM"))
        psA = ctx.enter_context(tc.tile_pool(name="psA", bufs=1, space="PSUM"))
        psB = ctx.enter_context(tc.tile_pool(name="psB", bufs=1, space="PSUM"))

        # ---- resident inputs (order = DMA priority) ----
        w012_t = const.tile([128, 8, 3, QL], F32)
        nc.sync.dma_start(out=w012_t, in_=w012_d.ap())
        x38_t = const.tile([128, 8, B, 38], BF16)
        nc.sync.dma_start(out=x38_t[:, :, 0:12, :],
                          in_=x38_d.ap()[:, :, 0:12, :])
        wct_t = const.tile([128, 8, D], BF16)
        nc.sync.dma_start(out=wct_t, in_=wct_d.ap())
        for bq in range(12, B, 12):
            nc.sync.dma_start(out=x38_t[:, :, bq:bq + 12, :],
                              in_=x38_d.ap()[:, :, bq:bq + 12, :])
        sel_t = const.tile([128, 2, NCH, 128], BF16)
        nc.sync.dma_start(out=sel_t, in_=sel_d.ap())
        bcb_t = const.tile([128, D], F32)
        nc.sync.dma_start(out=bcb_t, in_=bcb_d.ap())
        capn_t = const.tile([128, NPAIR, D], F32)
        nc.sync.dma_start(out=capn_t, in_=capn_d.ap())
        dot_t = const.tile([128, 2 * NPAIR], F32)

        def sel_mms(S, j, eb, pb, A_h, B_h):
            selj = sel_t[:, S, j, :]
            for h in range(2):
                hs = slice(h * 512, (h + 1) * 512)
                nc.tensor.matmul(A_h[h], lhsT=selj, rhs=eb[:, hs],
                                 start=(S == 0 and j == 0),
                                 stop=(S == 1 and j == NCH - 1))
                nc.tensor.matmul(B_h[h], lhsT=selj, rhs=pb[:, hs],
                                 start=(S == 0 and j == 0),
                                 stop=(S == 1 and j == NCH - 1))

        # xcv tiles, allocated lazily; X-stage emitted block-by-block so
        # caption q+1's tap chain interleaves into caption q's chunk loop
        # (engine FIFOs are in-order: without interleaving, q+1's DVE/ACT
        # work queues behind q's PSUM-gated p-passes and PE stalls at every
        # caption boundary)
        xcv_of = {}

        def get_xcv(q):
            if q not in xcv_of:
                xcv_of[q] = xcp.tile([128, 8, NP], BF16, tag="xc",
                                     name=f"xcv{q}")
                nc.vector.memset(xcv_of[q][:, :, NB:NP], 0.0)
            return xcv_of[q]

        def emit_x_block(q, blk, nb=24, act_assist=False):
            half, cc = blk >> 3, blk & 7
            xcvq = get_xcv(q)
            b0, b1 = half * nb, half * nb + nb
            xs = lambda o: x38_t[:, cc, b0:b1, o:o + 36]
            t0 = tap.tile([128, 24, 36], BF16, tag="t0", name="t0")[:, 0:nb, :]
            if act_assist or cc in T0_ACT:
                nc.scalar.mul(t0, xs(2), w012_t[:, cc, 2, q:q + 1])
            else:
                nc.vector.tensor_scalar_mul(t0, xs(2),
                                            w012_t[:, cc, 2, q:q + 1])
            u0 = tap.tile([128, 24, 36], BF16, tag="u0", name="u0")[:, 0:nb, :]
            if act_assist:
                nc.scalar.mul(u0, xs(1), w012_t[:, cc, 1, q:q + 1])
            else:
                nc.vector.tensor_scalar_mul(u0, xs(1),
                                            w012_t[:, cc, 1, q:q + 1])
            u1 = tap.tile([128, 24, 36], BF16, tag="u1", name="u1")[:, 0:nb, :]
            nc.vector.tensor_tensor(u1, u0, t0, OP.add)
            xo = xcvq[:, cc, b0 * 36:b1 * 36].rearrange(
                "p (b r) -> p b r", r=36)
            nc.vector.scalar_tensor_tensor(
                xo, xs(0), w012_t[:, cc, 0, q:q + 1], u1, OP.mult, OP.add)

        # first caption in b-quarters (quarter-major): quarters 0-1 up
        # front (enough for chunks 0-5), quarters 2-3 paced into the loop
        # so caption-0's p-passes aren't starved on DVE
        for blk in range(16):
            emit_x_block(0, blk, nb=12)

        lagq = []

        def drain_lag(n):
            while len(lagq) > n:
                s2, jj, e2, p2, a2, b2 = lagq.pop(0)
                sel_mms(s2, jj, e2, p2, a2, b2)

        for P in range(NPAIR):
            A_h = [psA.tile([128, 512], F32, tag=f"A{h}", name=f"A{h}")
                   for h in range(2)]
            B_h = [psB.tile([128, 512], F32, tag=f"B{h}", name=f"B{h}")
                   for h in range(2)]
            for S in range(2):
                q = 2 * P + S
                xcv = get_xcv(q)
                nxt = nxt0 = 0

                # ---- M/E/S over 14 n-chunks; selector MMs lag two chunks
                # (carried across caption bounds) so PE never waits on the
                # just-produced e/p tiles ----
                for j in range(NCH):
                    eb = ep.tile([128, D], BF16, tag="e", name="eb")
                    pb = ep.tile([128, D], BF16, tag="p", name="pb")
                    yb = ep.tile([128, D], BF16, tag="yb", name="yb")
                    for h in range(2):
                        hs = slice(h * 512, (h + 1) * 512)
                        y_ps = psy.tile([128, 512], F32, tag="y")
                        for cc in range(8):
                            nc.tensor.matmul(
                                y_ps,
                                lhsT=xcv[:, cc, j * 128:(j + 1) * 128],
                                rhs=wct_t[:, cc, hs],
                                start=(cc == 0), stop=(cc == 7))
                        # both PSUM consumers on ACT so the y buffer frees at
                        # ACT's pace; p then runs all-bf16 on DVE at 2x
                        nc.scalar.activation(eb[:, hs], y_ps, AF.Exp)
                        nc.scalar.copy(yb[:, hs], y_ps)
                        nc.vector.tensor_tensor(pb[:, hs], yb[:, hs],
                                                eb[:, hs], OP.mult)
                    lagq.append((S, j, eb, pb, A_h, B_h))
                    drain_lag(2)
                    # finish caption-0's own quarters 2-3 early
                    if q == 0:
                        while nxt0 < min(16, (j + 1) * 3):
                            emit_x_block(0, 16 + nxt0, nb=12)
                            nxt0 += 1
                    # pace next caption's tap chain into this loop
                    if q + 1 < QL:
                        while nxt < (j + 1) * 16 // NCH:
                            emit_x_block(q + 1, nxt)
                            nxt += 1
            # pair's accumulation must close before its epilogue
            drain_lag(0)

            # ---- epilogue per caption pair (f32) ----
            # 1/A via exp(-ln(A)) on ScalarE (A > 0): custom-DVE reciprocal
            # is unsupported on this runtime
            rA = epi.tile([128, D], F32, tag="rA")
            vb = epi.tile([128, D], F32, tag="vb")
            for h in range(2):
                hs = slice(h * 512, (h + 1) * 512)
                nc.scalar.activation(rA[:, hs], A_h[h], AF.Ln)
                nc.scalar.activation(rA[:, hs], rA[:, hs], AF.Exp,
                                     scale=-1.0)
                nc.vector.scalar_tensor_tensor(vb[:, hs], B_h[h], 1.0,
                                               rA[:, hs], OP.mult, OP.mult)
            nc.vector.tensor_tensor(vb, vb, bcb_t, OP.add)
            scr = epi.tile([128, D], F32, tag="scr")
            nc.vector.tensor_tensor(scr, vb, capn_t[:, P, :], OP.mult)
            nc.vector.tensor_reduce(dot_t[:, P:P + 1], scr,
                                    mybir.AxisListType.X, OP.add)
            nc.vector.tensor_tensor(scr, vb, vb, OP.mult)
            nc.vector.tensor_reduce(dot_t[:, NPAIR + P:NPAIR + P + 1], scr,
                                    mybir.AxisListType.X, OP.add)

        nc.sync.dma_start(out=out_d.ap(), in_=dot_t)

    nc.compile()
    return nc


def _host_prep(img, cap0, Wred, bred, Wproj, bproj, Wconv):
    """Shared (caption-independent) tensors + per-caption tap weights."""
    capr = cap0 @ Wred.T + bred
    wdyn = (capr @ Wproj.T + bproj).reshape(Q, D, 3)
    wdyn = np.exp(wdyn - wdyn.max(-1, keepdims=True))
    wdyn /= wdyn.sum(-1, keepdims=True)                    # (Q, D, 3)

    xt = img.transpose(2, 0, 1)                            # (D, B, R)
    x38 = np.zeros((D, B, 38), np.float32)
    x38[:, :, 1:37] = xt
    x38 = np.ascontiguousarray(
        x38.reshape(8, 128, B, 38).transpose(1, 0, 2, 3)).astype(NPBF)

    # wct[p, cc, d] = Wconv[d, c], c = 128 cc + p
    wct = np.ascontiguousarray(
        Wconv.T.reshape(8, 128, D).transpose(1, 0, 2)).astype(NPBF)

    # 0/1 selectors: S=0 -> out cols 0..63, S=1 -> 64..127
    sel = np.zeros((128, 2, NCH, 128), np.float32)
    n = np.arange(NB)
    p, ch = n % 128, n // 128
    for s in range(2):
        sel[p, s, ch, 64 * s + n // R] = 1.0
        # pad out-rows (b 48..63 per slot) tied to invalid-n rows of chunk 13
        # (xcv zeros there -> e = 1, p = 0) so A accumulates 1.0 and the
        # epilogue reciprocal stays finite
        for t in range(16):
            sel[64 + t, s, NCH - 1, 64 * s + 48 + t] = 1.0
    return wdyn, x38, wct, sel.astype(NPBF)


def kernel(img_embed, cap_embed, lens, Wred, bred, Wproj, bproj, Wconv,
           bconv, **_unused):
    global LAST_EXEC_NS
    img_embed = np.asarray(img_embed, np.float32)
    cap0 = np.asarray(cap_embed, np.float32)[:, 0, :]
    Wred = np.asarray(Wred, np.float32)
    bred_a = np.asarray(bred, np.float32)
    Wproj = np.asarray(Wproj, np.float32)
    bproj_a = np.asarray(bproj, np.float32)
    Wconv = np.asarray(Wconv, np.float32)
    bconv_a = np.asarray(bconv, np.float32)

    if "nc" not in _CACHE:
        _CACHE["nc"] = _build_nc()
    nc = _CACHE["nc"]

    wdyn, x38, wct, sel = _host_prep(img_embed, cap0, Wred, bred_a,
                                     Wproj, bproj_a, Wconv)
    capn = cap0 / np.linalg.norm(cap0, axis=1, keepdims=True)   # (Q, D)
    bcb = np.ascontiguousarray(
        np.broadcast_to(bconv_a, (128, D))).astype(np.float32)

    in_maps = []
    for c in range(N_CORES):
        qs = slice(c * QL, (c + 1) * QL)
        # w012[p, cc, tap, q] = wdyn[q_local, 128 cc + p, tap]
        w012 = np.ascontiguousarray(
            wdyn[qs].reshape(QL, 8, 128, 3).transpose(2, 1, 3, 0)
        ).astype(np.float32)
        # capn pairs: partitions 0..63 even caption, 64..127 odd
        cpn = np.empty((128, NPAIR, D), np.float32)
        for P in range(NPAIR):
            cpn[0:64, P] = capn[c * QL + 2 * P]
            cpn[64:128, P] = capn[c * QL + 2 * P + 1]
        in_maps.append({
            "x38": x38, "wct": wct, "sel": sel, "w012": w012,
            "capn": cpn, "bcb": bcb,
        })

    trace = bool(int(os.environ.get("KTRACE", "0")))
    tdir = os.environ.get("KTRACE_DIR") or None
    res = run_bass_kernel_spmd(nc, in_maps, core_ids=list(range(N_CORES)),
                               trace=trace, tmpdir=tdir)
    LAST_EXEC_NS = res.exec_time_ns

    sims = np.empty((B, Q), np.float32)
    for c in range(N_CORES):
        o = np.asarray(res.results[c]["out"], np.float32)   # (128, 6)
        for P in range(NPAIR):
            for S in range(2):
                rows = slice(64 * S, 64 * S + 48)
                dot = o[rows, P]
                s2 = o[rows, NPAIR + P]
                sims[:, c * QL + 2 * P + S] = dot / np.sqrt(s2)
    return sims


# revision 5
# speedup vs baseline: 1.0503x; 1.0033x over previous
"""Trainium2 Bass kernel for nn_KernelProjectionT2I — v2.

Sharding: data-parallel over captions (B_cap=48 -> 6 per core on 8 cores).
Each core holds the full image batch + conv weights and computes its 6
similarity columns; the host concatenates.

Host precompute (cheap, O(Q*D) math):
  caption MLP + tap softmax -> per-caption tap weights w0/w1/w2 (Q, D)
  cap l2-normalization; final sims = dot / sqrt(s2) division.

Device math per caption q (layout: channels c on partitions for the tap
chain, image-region pixels n=(b,r) on PSUM partitions for y):
  xcv = bf16(w0*x[r-1] + w1*x[r] + w2*x[r+1])     tap chain (DVE)
  y   = Wconv @ xcv                                bf16 matmuls, f32 PSUM
  e   = bf16(exp(y)); yb = bf16(y)                 ACT (frees PSUM fast)
  p   = bf16(yb * e)                               DVE 2x (all-bf16 SBUF)
  A   = sum_r e ; Bs = sum_r p                     0/1-selector bf16 matmuls
                                                   (caption pair packed into
                                                   64+64 output partitions)
  v   = Bs/A + bconv                               f32 epilogue (DVE)
  dot = <v, capn>; s2 = <v, v>                     fused TT-reduce

Outputs per core: dot/s2 [128, 3+3] f32; host finishes sims = dot/sqrt(s2).
"""

import os
import numpy as np
import ml_dtypes
from contextlib import ExitStack

import concourse.bass as bass
import concourse.tile as tile
from concourse import bacc, mybir
from concourse.bass_utils import run_bass_kernel_spmd

F32 = mybir.dt.float32
BF16 = mybir.dt.bfloat16
AF = mybir.ActivationFunctionType
OP = mybir.AluOpType

NPBF = ml_dtypes.bfloat16

N_CORES = 8
B, R, D = 48, 36, 1024
Q = 48
QL = Q // N_CORES            # 6 captions per core
NPAIR = QL // 2              # 3 caption pairs
NB = B * R                   # 1728
NP = 1792                    # padded n, 14 chunks of 128
NCH = NP // 128              # 14

# cc's whose tap2 (t0) multiply runs on ACT (rest DVE tensor_scalar)
T0_ACT = frozenset()

LAST_EXEC_NS = None
_CACHE = {}


def _build_nc():
    nc = bacc.Bacc(trn_type="TRN2", target_bir_lowering=False,
                   num_devices=N_CORES)
    x38_d = nc.dram_tensor("x38", [128, 8, B, 38], BF16, kind="ExternalInput")
    wct_d = nc.dram_tensor("wct", [128, 8, D], BF16, kind="ExternalInput")
    sel_d = nc.dram_tensor("sel", [128, 2, NCH, 128], BF16,
                           kind="ExternalInput")
    w012_d = nc.dram_tensor("w012", [128, 8, 3, QL], F32,
                            kind="ExternalInput")
    capn_d = nc.dram_tensor("capn", [128, NPAIR, D], F32,
                            kind="ExternalInput")
    bcb_d = nc.dram_tensor("bcb", [128, D], F32, kind="ExternalInput")
    out_d = nc.dram_tensor("out", [128, 2 * NPAIR], F32, kind="ExternalOutput")

    with ExitStack() as ctx:
        tc = ctx.enter_context(tile.TileContext(nc))
        const = ctx.enter_context(tc.tile_pool(name="const", bufs=1))
        tap = ctx.enter_context(tc.tile_pool(name="tap", bufs=3))
        xcp = ctx.enter_context(tc.tile_pool(name="xcp", bufs=2))
        ep = ctx.enter_context(tc.tile_pool(name="ep", bufs=4))
        epi = ctx.enter_context(tc.tile_pool(name="epi", bufs=2))
        psy = ctx.enter_context(tc.tile_pool(name="psy", bufs=4, space="PSUM"))
        psA = ctx.enter_context(tc.tile_pool(name="psA", bufs=1, space="PSUM"))
        psB = ctx.enter_context(tc.tile_pool(name="psB", bufs=1, space="PSUM"))

        # ---- resident inputs (order = DMA priority) ----
        w012_t = const.tile([128, 8, 3, QL], F32)
        nc.sync.dma_start(out=w012_t, in_=w012_d.ap())
        x38_t = const.tile([128, 8, B, 38], BF16)
        nc.sync.dma_start(out=x38_t[:, :, 0:12, :],
                          in_=x38_d.ap()[:, :, 0:12, :])
        wct_t = const.tile([128, 8, D], BF16)
        nc.sync.dma_start(out=wct_t, in_=wct_d.ap())
        for bq in range(12, B, 12):
            nc.sync.dma_start(out=x38_t[:, :, bq:bq + 12, :],
                              in_=x38_d.ap()[:, :, bq:bq + 12, :])
        sel_t = const.tile([128, 2, NCH, 128], BF16)
        nc.sync.dma_start(out=sel_t, in_=sel_d.ap())
        bcb_t = const.tile([128, D], F32)
        nc.sync.dma_start(out=bcb_t, in_=bcb_d.ap())
        capn_t = const.tile([128, NPAIR, D], F32)
        nc.sync.dma_start(out=capn_t, in_=capn_d.ap())
        dot_t = const.tile([128, 2 * NPAIR], F32)

        def sel_mms(S, j, eb, pb, A_h, B_h):
            selj = sel_t[:, S, j, :]
            for h in range(2):
                hs = slice(h * 512, (h + 1) * 512)
                nc.tensor.matmul(A_h[h], lhsT=selj, rhs=eb[:, hs],
                                 start=(S == 0 and j == 0),
                                 stop=(S == 1 and j == NCH - 1))
                nc.tensor.matmul(B_h[h], lhsT=selj, rhs=pb[:, hs],
                                 start=(S == 0 and j == 0),
                                 stop=(S == 1 and j == NCH - 1))

        # xcv tiles, allocated lazily; X-stage emitted block-by-block so
        # caption q+1's tap chain interleaves into caption q's chunk loop
        # (engine FIFOs are in-order: without interleaving, q+1's DVE/ACT
        # work queues behind q's PSUM-gated p-passes and PE stalls at every
        # caption boundary)
        xcv_of = {}

        def get_xcv(q):
            if q not in xcv_of:
                xcv_of[q] = xcp.tile([128, 8, NP], BF16, tag="xc",
                                     name=f"xcv{q}")
                nc.vector.memset(xcv_of[q][:, :, NB:NP], 0.0)
            return xcv_of[q]

        def emit_x_block(q, blk, nb=24, act_assist=False):
            half, cc = blk >> 3, blk & 7
            xcvq = get_xcv(q)
            b0, b1 = half * nb, half * nb + nb
            xs = lambda o: x38_t[:, cc, b0:b1, o:o + 36]
            t0 = tap.tile([128, 24, 36], BF16, tag="t0", name="t0")[:, 0:nb, :]
            if act_assist or cc in T0_ACT:
                nc.scalar.mul(t0, xs(2), w012_t[:, cc, 2, q:q + 1])
            else:
                nc.vector.tensor_scalar_mul(t0, xs(2),
                                            w012_t[:, cc, 2, q:q + 1])
            u0 = tap.tile([128, 24, 36], BF16, tag="u0", name="u0")[:, 0:nb, :]
            if act_assist:
                nc.scalar.mul(u0, xs(1), w012_t[:, cc, 1, q:q + 1])
            else:
                nc.vector.tensor_scalar_mul(u0, xs(1),
                                            w012_t[:, cc, 1, q:q + 1])
            u1 = tap.tile([128, 24, 36], BF16, tag="u1", name="u1")[:, 0:nb, :]
            nc.vector.tensor_tensor(u1, u0, t0, OP.add)
            xo = xcvq[:, cc, b0 * 36:b1 * 36].rearrange(
                "p (b r) -> p b r", r=36)
            nc.vector.scalar_tensor_tensor(
                xo, xs(0), w012_t[:, cc, 0, q:q + 1], u1, OP.mult, OP.add)

        # first caption in b-quarters (quarter-major): quarters 0-1 up
        # front (enough for chunks 0-5), quarters 2-3 paced into the loop
        # so caption-0's p-passes aren't starved on DVE
        for blk in range(16):
            emit_x_block(0, blk, nb=12, act_assist=(blk < 8))

        def emit_epilogue(P, A_h, B_h):
            # 1/A via exp(-ln(A)) on ScalarE (A > 0): custom-DVE reciprocal
            # is unsupported on this runtime
            rA = epi.tile([128, D], F32, tag="rA", name="rA")
            vb = epi.tile([128, D], F32, tag="vb", name="vb")
            for h in range(2):
                hs = slice(h * 512, (h + 1) * 512)
                nc.scalar.activation(rA[:, hs], A_h[h], AF.Ln)
                nc.scalar.activation(rA[:, hs], rA[:, hs], AF.Exp,
                                     scale=-1.0)
                nc.vector.scalar_tensor_tensor(vb[:, hs], B_h[h], 1.0,
                                               rA[:, hs], OP.mult, OP.mult)
            nc.vector.tensor_tensor(vb, vb, bcb_t, OP.add)
            scr = epi.tile([128, D], F32, tag="scr", name="scr")
            sq = epi.tile([128, D], F32, tag="sq", name="sq")
            nc.scalar.square(sq, vb)
            nc.vector.tensor_tensor(scr, vb, capn_t[:, P, :], OP.mult)
            nc.vector.tensor_reduce(dot_t[:, P:P + 1], scr,
                                    mybir.AxisListType.X, OP.add)
            nc.vector.tensor_reduce(dot_t[:, NPAIR + P:NPAIR + P + 1], sq,
                                    mybir.AxisListType.X, OP.add)

        lagq = []
        pending_epi = None

        def drain_lag(n):
            while len(lagq) > n:
                s2, jj, e2, p2, a2, b2 = lagq.pop(0)
                sel_mms(s2, jj, e2, p2, a2, b2)

        for P in range(NPAIR):
            A_h = [psA.tile([128, 512], F32, tag=f"A{h}", name=f"A{h}")
                   for h in range(2)]
            B_h = [psB.tile([128, 512], F32, tag=f"B{h}", name=f"B{h}")
                   for h in range(2)]
            for S in range(2):
                q = 2 * P + S
                xcv = get_xcv(q)
                nxt = nxt0 = 0

                # ---- M/E/S over 14 n-chunks; selector MMs lag two chunks
                # (carried across caption bounds) so PE never waits on the
                # just-produced e/p tiles ----
                for j in range(NCH):
                    eb = ep.tile([128, D], BF16, tag="e", name="eb")
                    pb = ep.tile([128, D], BF16, tag="p", name="pb")
                    yb = ep.tile([128, D], BF16, tag="yb", name="yb")
                    for h in range(2):
                        hs = slice(h * 512, (h + 1) * 512)
                        y_ps = psy.tile([128, 512], F32, tag="y")
                        for cc in range(8):
                            nc.tensor.matmul(
                                y_ps,
                                lhsT=xcv[:, cc, j * 128:(j + 1) * 128],
                                rhs=wct_t[:, cc, hs],
                                start=(cc == 0), stop=(cc == 7))
                        # both PSUM consumers on ACT so the y buffer frees at
                        # ACT's pace; p then runs all-bf16 on DVE at 2x
                        nc.scalar.activation(eb[:, hs], y_ps, AF.Exp)
                        nc.scalar.copy(yb[:, hs], y_ps)
                        nc.vector.tensor_tensor(pb[:, hs], yb[:, hs],
                                                eb[:, hs], OP.mult)
                    lagq.append((S, j, eb, pb, A_h, B_h))
                    drain_lag(1 if (S == 1 and j == NCH - 1) else 2)
                    if j == 1 and pending_epi is not None:
                        emit_epilogue(*pending_epi)
                        pending_epi = None
                    # finish caption-0's own quarters 2-3 early
                    if q == 0:
                        while nxt0 < min(16, (j + 1) * 3):
                            emit_x_block(0, 16 + nxt0, nb=12)
                            nxt0 += 1
                    # pace next caption's tap chain into this loop
                    if q + 1 < QL:
                        while nxt < (j + 1) * 16 // NCH:
                            emit_x_block(q + 1, nxt)
                            nxt += 1
            # pair's accumulation must close before its epilogue
            drain_lag(0)
            pending_epi = (P, A_h, B_h)

        emit_epilogue(*pending_epi)
        nc.sync.dma_start(out=out_d.ap(), in_=dot_t)

    nc.compile()
    return nc


def _host_prep(img, cap0, Wred, bred, Wproj, bproj, Wconv):
    """Shared (caption-independent) tensors + per-caption tap weights."""
    capr = cap0 @ Wred.T + bred
    wdyn = (capr @ Wproj.T + bproj).reshape(Q, D, 3)
    wdyn = np.exp(wdyn - wdyn.max(-1, keepdims=True))
    wdyn /= wdyn.sum(-1, keepdims=True)                    # (Q, D, 3)

    xt = img.transpose(2, 0, 1)                            # (D, B, R)
    x38 = np.zeros((D, B, 38), np.float32)
    x38[:, :, 1:37] = xt
    x38 = np.ascontiguousarray(
        x38.reshape(8, 128, B, 38).transpose(1, 0, 2, 3)).astype(NPBF)

    # wct[p, cc, d] = Wconv[d, c], c = 128 cc + p
    wct = np.ascontiguousarray(
        Wconv.T.reshape(8, 128, D).transpose(1, 0, 2)).astype(NPBF)

    # 0/1 selectors: S=0 -> out cols 0..63, S=1 -> 64..127
    sel = np.zeros((128, 2, NCH, 128), np.float32)
    n = np.arange(NB)
    p, ch = n % 128, n // 128
    for s in range(2):
        sel[p, s, ch, 64 * s + n // R] = 1.0
        # pad out-rows (b 48..63 per slot) tied to invalid-n rows of chunk 13
        # (xcv zeros there -> e = 1, p = 0) so A accumulates 1.0 and the
        # epilogue reciprocal stays finite
        for t in range(16):
            sel[64 + t, s, NCH - 1, 64 * s + 48 + t] = 1.0
    return wdyn, x38, wct, sel.astype(NPBF)


def kernel(img_embed, cap_embed, lens, Wred, bred, Wproj, bproj, Wconv,
           bconv, **_unused):
    global LAST_EXEC_NS
    img_embed = np.asarray(img_embed, np.float32)
    cap0 = np.asarray(cap_embed, np.float32)[:, 0, :]
    Wred = np.asarray(Wred, np.float32)
    bred_a = np.asarray(bred, np.float32)
    Wproj = np.asarray(Wproj, np.float32)
    bproj_a = np.asarray(bproj, np.float32)
    Wconv = np.asarray(Wconv, np.float32)
    bconv_a = np.asarray(bconv, np.float32)

    if "nc" not in _CACHE:
        _CACHE["nc"] = _build_nc()
    nc = _CACHE["nc"]

    wdyn, x38, wct, sel = _host_prep(img_embed, cap0, Wred, bred_a,
                                     Wproj, bproj_a, Wconv)
    capn = cap0 / np.linalg.norm(cap0, axis=1, keepdims=True)   # (Q, D)
    bcb = np.ascontiguousarray(
        np.broadcast_to(bconv_a, (128, D))).astype(np.float32)

    in_maps = []
    for c in range(N_CORES):
        qs = slice(c * QL, (c + 1) * QL)
        # w012[p, cc, tap, q] = wdyn[q_local, 128 cc + p, tap]
        w012 = np.ascontiguousarray(
            wdyn[qs].reshape(QL, 8, 128, 3).transpose(2, 1, 3, 0)
        ).astype(np.float32)
        # capn pairs: partitions 0..63 even caption, 64..127 odd
        cpn = np.empty((128, NPAIR, D), np.float32)
        for P in range(NPAIR):
            cpn[0:64, P] = capn[c * QL + 2 * P]
            cpn[64:128, P] = capn[c * QL + 2 * P + 1]
        in_maps.append({
            "x38": x38, "wct": wct, "sel": sel, "w012": w012,
            "capn": cpn, "bcb": bcb,
        })

    trace = bool(int(os.environ.get("KTRACE", "0")))
    tdir = os.environ.get("KTRACE_DIR") or None
    res = run_bass_kernel_spmd(nc, in_maps, core_ids=list(range(N_CORES)),
                               trace=trace, tmpdir=tdir)
    LAST_EXEC_NS = res.exec_time_ns

    sims = np.empty((B, Q), np.float32)
    for c in range(N_CORES):
        o = np.asarray(res.results[c]["out"], np.float32)   # (128, 6)
        for P in range(NPAIR):
            for S in range(2):
                rows = slice(64 * S, 64 * S + 48)
                dot = o[rows, P]
                s2 = o[rows, NPAIR + P]
                sims[:, c * QL + 2 * P + S] = dot / np.sqrt(s2)
    return sims


# revision 6
# speedup vs baseline: 1.0532x; 1.0027x over previous
"""Trainium2 Bass kernel for nn_KernelProjectionT2I — v2.

Sharding: data-parallel over captions (B_cap=48 -> 6 per core on 8 cores).
Each core holds the full image batch + conv weights and computes its 6
similarity columns; the host concatenates.

Host precompute (cheap, O(Q*D) math):
  caption MLP + tap softmax -> per-caption tap weights w0/w1/w2 (Q, D)
  cap l2-normalization; final sims = dot / sqrt(s2) division.

Device math per caption q (layout: channels c on partitions for the tap
chain, image-region pixels n=(b,r) on PSUM partitions for y):
  xcv = bf16(w0*x[r-1] + w1*x[r] + w2*x[r+1])     tap chain (DVE)
  y   = Wconv @ xcv                                bf16 matmuls, f32 PSUM
  e   = bf16(exp(y)); yb = bf16(y)                 ACT (frees PSUM fast)
  p   = bf16(yb * e)                               DVE 2x (all-bf16 SBUF)
  A   = sum_r e ; Bs = sum_r p                     0/1-selector bf16 matmuls
                                                   (caption pair packed into
                                                   64+64 output partitions)
  v   = Bs/A + bconv                               f32 epilogue (DVE)
  dot = <v, capn>; s2 = <v, v>                     fused TT-reduce

Outputs per core: dot/s2 [128, 3+3] f32; host finishes sims = dot/sqrt(s2).
"""

import os
import numpy as np
import ml_dtypes
from contextlib import ExitStack

import concourse.bass as bass
import concourse.tile as tile
from concourse import bacc, mybir
from concourse.bass_utils import run_bass_kernel_spmd

F32 = mybir.dt.float32
BF16 = mybir.dt.bfloat16
AF = mybir.ActivationFunctionType
OP = mybir.AluOpType

NPBF = ml_dtypes.bfloat16

N_CORES = 8
B, R, D = 48, 36, 1024
Q = 48
QL = Q // N_CORES            # 6 captions per core
NPAIR = QL // 2              # 3 caption pairs
NB = B * R                   # 1728
NP = 1792                    # padded n, 14 chunks of 128
NCH = NP // 128              # 14

# cc's whose tap2 (t0) multiply runs on ACT (rest DVE tensor_scalar)
T0_ACT = frozenset()

LAST_EXEC_NS = None
_CACHE = {}


def _build_nc():
    nc = bacc.Bacc(trn_type="TRN2", target_bir_lowering=False,
                   num_devices=N_CORES)
    x38_d = nc.dram_tensor("x38", [128, 8, B, 38], BF16, kind="ExternalInput")
    wct_d = nc.dram_tensor("wct", [128, 8, D], BF16, kind="ExternalInput")
    sel_d = nc.dram_tensor("sel", [128, 2, NCH, 128], BF16,
                           kind="ExternalInput")
    w012_d = nc.dram_tensor("w012", [128, 8, 3, QL], F32,
                            kind="ExternalInput")
    capn_d = nc.dram_tensor("capn", [128, NPAIR, D], F32,
                            kind="ExternalInput")
    bcb_d = nc.dram_tensor("bcb", [128, D], F32, kind="ExternalInput")
    out_d = nc.dram_tensor("out", [128, 2 * NPAIR], F32, kind="ExternalOutput")

    with ExitStack() as ctx:
        tc = ctx.enter_context(tile.TileContext(nc))
        const = ctx.enter_context(tc.tile_pool(name="const", bufs=1))
        tap = ctx.enter_context(tc.tile_pool(name="tap", bufs=3))
        xcp = ctx.enter_context(tc.tile_pool(name="xcp", bufs=2))
        ep = ctx.enter_context(tc.tile_pool(name="ep", bufs=6))
        epi = ctx.enter_context(tc.tile_pool(name="epi", bufs=2))
        psy = ctx.enter_context(tc.tile_pool(name="psy", bufs=4, space="PSUM"))
        psA = ctx.enter_context(tc.tile_pool(name="psA", bufs=1, space="PSUM"))
        psB = ctx.enter_context(tc.tile_pool(name="psB", bufs=1, space="PSUM"))

        # ---- resident inputs (order = DMA priority) ----
        w012_t = const.tile([128, 8, 3, QL], F32)
        nc.sync.dma_start(out=w012_t, in_=w012_d.ap())
        x38_t = const.tile([128, 8, B, 38], BF16)
        nc.sync.dma_start(out=x38_t[:, :, 0:12, :],
                          in_=x38_d.ap()[:, :, 0:12, :])
        wct_t = const.tile([128, 8, D], BF16)
        nc.sync.dma_start(out=wct_t[:, :, 0:512], in_=wct_d.ap()[:, :, 0:512])
        nc.sync.dma_start(out=wct_t[:, :, 512:D],
                          in_=wct_d.ap()[:, :, 512:D])
        for bq in range(12, B, 12):
            nc.sync.dma_start(out=x38_t[:, :, bq:bq + 12, :],
                              in_=x38_d.ap()[:, :, bq:bq + 12, :])
        sel_t = const.tile([128, 2, NCH, 128], BF16)
        nc.sync.dma_start(out=sel_t, in_=sel_d.ap())
        bcb_t = const.tile([128, D], F32)
        nc.sync.dma_start(out=bcb_t, in_=bcb_d.ap())
        capn_t = const.tile([128, NPAIR, D], F32)
        nc.sync.dma_start(out=capn_t, in_=capn_d.ap())
        dot_t = const.tile([128, 2 * NPAIR], F32)

        def sel_mms(S, j, eb, pb, A_h, B_h):
            selj = sel_t[:, S, j, :]
            for h in range(2):
                hs = slice(h * 512, (h + 1) * 512)
                nc.tensor.matmul(A_h[h], lhsT=selj, rhs=eb[:, hs],
                                 start=(S == 0 and j == 0),
                                 stop=(S == 1 and j == NCH - 1))
                nc.tensor.matmul(B_h[h], lhsT=selj, rhs=pb[:, hs],
                                 start=(S == 0 and j == 0),
                                 stop=(S == 1 and j == NCH - 1))

        # xcv tiles, allocated lazily; X-stage emitted block-by-block so
        # caption q+1's tap chain interleaves into caption q's chunk loop
        # (engine FIFOs are in-order: without interleaving, q+1's DVE/ACT
        # work queues behind q's PSUM-gated p-passes and PE stalls at every
        # caption boundary)
        xcv_of = {}

        def get_xcv(q):
            if q not in xcv_of:
                xcv_of[q] = xcp.tile([128, 8, NP], BF16, tag="xc",
                                     name=f"xcv{q}")
                nc.vector.memset(xcv_of[q][:, :, NB:NP], 0.0)
            return xcv_of[q]

        def emit_x_block(q, blk, nb=24, act_assist=False):
            half, cc = blk >> 3, blk & 7
            xcvq = get_xcv(q)
            b0, b1 = half * nb, half * nb + nb
            xs = lambda o: x38_t[:, cc, b0:b1, o:o + 36]
            t0 = tap.tile([128, 24, 36], BF16, tag="t0", name="t0")[:, 0:nb, :]
            if act_assist or cc in T0_ACT:
                nc.scalar.mul(t0, xs(2), w012_t[:, cc, 2, q:q + 1])
            else:
                nc.vector.tensor_scalar_mul(t0, xs(2),
                                            w012_t[:, cc, 2, q:q + 1])
            u0 = tap.tile([128, 24, 36], BF16, tag="u0", name="u0")[:, 0:nb, :]
            if act_assist:
                nc.scalar.mul(u0, xs(1), w012_t[:, cc, 1, q:q + 1])
            else:
                nc.vector.tensor_scalar_mul(u0, xs(1),
                                            w012_t[:, cc, 1, q:q + 1])
            u1 = tap.tile([128, 24, 36], BF16, tag="u1", name="u1")[:, 0:nb, :]
            nc.vector.tensor_tensor(u1, u0, t0, OP.add)
            xo = xcvq[:, cc, b0 * 36:b1 * 36].rearrange(
                "p (b r) -> p b r", r=36)
            nc.vector.scalar_tensor_tensor(
                xo, xs(0), w012_t[:, cc, 0, q:q + 1], u1, OP.mult, OP.add)

        # first caption in b-quarters (quarter-major): quarters 0-1 up
        # front (enough for chunks 0-5), quarters 2-3 paced into the loop
        # so caption-0's p-passes aren't starved on DVE
        for blk in range(16):
            emit_x_block(0, blk, nb=12, act_assist=(blk < 8))

        def emit_epilogue(P, A_h, B_h):
            # 1/A via exp(-ln(A)) on ScalarE (A > 0): custom-DVE reciprocal
            # is unsupported on this runtime
            rA = epi.tile([128, D], F32, tag="rA", name="rA")
            vb = epi.tile([128, D], F32, tag="vb", name="vb")
            for h in range(2):
                hs = slice(h * 512, (h + 1) * 512)
                nc.scalar.activation(rA[:, hs], A_h[h], AF.Ln)
                nc.scalar.activation(rA[:, hs], rA[:, hs], AF.Exp,
                                     scale=-1.0)
                nc.vector.scalar_tensor_tensor(vb[:, hs], B_h[h], 1.0,
                                               rA[:, hs], OP.mult, OP.mult)
            nc.vector.tensor_tensor(vb, vb, bcb_t, OP.add)
            scr = epi.tile([128, D], F32, tag="scr", name="scr")
            sq = epi.tile([128, D], F32, tag="sq", name="sq")
            nc.scalar.square(sq, vb)
            nc.vector.tensor_tensor(scr, vb, capn_t[:, P, :], OP.mult)
            nc.vector.tensor_reduce(dot_t[:, P:P + 1], scr,
                                    mybir.AxisListType.X, OP.add)
            nc.vector.tensor_reduce(dot_t[:, NPAIR + P:NPAIR + P + 1], sq,
                                    mybir.AxisListType.X, OP.add)

        lagq = []
        pending_epi = None

        def drain_lag(n):
            while len(lagq) > n:
                s2, jj, e2, p2, a2, b2 = lagq.pop(0)
                sel_mms(s2, jj, e2, p2, a2, b2)

        for P in range(NPAIR):
            A_h = [psA.tile([128, 512], F32, tag=f"A{h}", name=f"A{h}")
                   for h in range(2)]
            B_h = [psB.tile([128, 512], F32, tag=f"B{h}", name=f"B{h}")
                   for h in range(2)]
            for S in range(2):
                q = 2 * P + S
                xcv = get_xcv(q)
                nxt = nxt0 = 0

                # ---- M/E/S over 14 n-chunks; selector MMs lag two chunks
                # (carried across caption bounds) so PE never waits on the
                # just-produced e/p tiles ----
                for j in range(NCH):
                    eb = ep.tile([128, D], BF16, tag="e", name="eb")
                    pb = ep.tile([128, D], BF16, tag="p", name="pb")
                    yb = ep.tile([128, D], BF16, tag="yb", name="yb")
                    for h in range(2):
                        hs = slice(h * 512, (h + 1) * 512)
                        y_ps = psy.tile([128, 512], F32, tag="y")
                        for cc in range(8):
                            nc.tensor.matmul(
                                y_ps,
                                lhsT=xcv[:, cc, j * 128:(j + 1) * 128],
                                rhs=wct_t[:, cc, hs],
                                start=(cc == 0), stop=(cc == 7))
                        # both PSUM consumers on ACT so the y buffer frees at
                        # ACT's pace; p then runs all-bf16 on DVE at 2x
                        nc.scalar.activation(eb[:, hs], y_ps, AF.Exp)
                        nc.scalar.copy(yb[:, hs], y_ps)
                        nc.vector.tensor_tensor(pb[:, hs], yb[:, hs],
                                                eb[:, hs], OP.mult)
                    lagq.append((S, j, eb, pb, A_h, B_h))
                    if P == 0 and S == 0 and j < 6:
                        depth = 4
                    elif S == 1 and j == NCH - 1:
                        depth = 1
                    else:
                        depth = 2
                    drain_lag(depth)
                    if j == 1 and pending_epi is not None:
                        emit_epilogue(*pending_epi)
                        pending_epi = None
                    # finish caption-0's own quarters 2-3 early
                    if q == 0:
                        while nxt0 < min(16, (j + 1) * 3):
                            emit_x_block(0, 16 + nxt0, nb=12)
                            nxt0 += 1
                    # pace next caption's tap chain into this loop
                    if q + 1 < QL:
                        while nxt < (j + 1) * 16 // NCH:
                            emit_x_block(q + 1, nxt)
                            nxt += 1
            # pair's accumulation must close before its epilogue
            drain_lag(0)
            pending_epi = (P, A_h, B_h)

        emit_epilogue(*pending_epi)
        nc.sync.dma_start(out=out_d.ap(), in_=dot_t)

    nc.compile()
    return nc


def _host_prep(img, cap0, Wred, bred, Wproj, bproj, Wconv):
    """Shared (caption-independent) tensors + per-caption tap weights."""
    capr = cap0 @ Wred.T + bred
    wdyn = (capr @ Wproj.T + bproj).reshape(Q, D, 3)
    wdyn = np.exp(wdyn - wdyn.max(-1, keepdims=True))
    wdyn /= wdyn.sum(-1, keepdims=True)                    # (Q, D, 3)

    xt = img.transpose(2, 0, 1)                            # (D, B, R)
    x38 = np.zeros((D, B, 38), np.float32)
    x38[:, :, 1:37] = xt
    x38 = np.ascontiguousarray(
        x38.reshape(8, 128, B, 38).transpose(1, 0, 2, 3)).astype(NPBF)

    # wct[p, cc, d] = Wconv[d, c], c = 128 cc + p
    wct = np.ascontiguousarray(
        Wconv.T.reshape(8, 128, D).transpose(1, 0, 2)).astype(NPBF)

    # 0/1 selectors: S=0 -> out cols 0..63, S=1 -> 64..127
    sel = np.zeros((128, 2, NCH, 128), np.float32)
    n = np.arange(NB)
    p, ch = n % 128, n // 128
    for s in range(2):
        sel[p, s, ch, 64 * s + n // R] = 1.0
        # pad out-rows (b 48..63 per slot) tied to invalid-n rows of chunk 13
        # (xcv zeros there -> e = 1, p = 0) so A accumulates 1.0 and the
        # epilogue reciprocal stays finite
        for t in range(16):
            sel[64 + t, s, NCH - 1, 64 * s + 48 + t] = 1.0
    return wdyn, x38, wct, sel.astype(NPBF)


def kernel(img_embed, cap_embed, lens, Wred, bred, Wproj, bproj, Wconv,
           bconv, **_unused):
    global LAST_EXEC_NS
    img_embed = np.asarray(img_embed, np.float32)
    cap0 = np.asarray(cap_embed, np.float32)[:, 0, :]
    Wred = np.asarray(Wred, np.float32)
    bred_a = np.asarray(bred, np.float32)
    Wproj = np.asarray(Wproj, np.float32)
    bproj_a = np.asarray(bproj, np.float32)
    Wconv = np.asarray(Wconv, np.float32)
    bconv_a = np.asarray(bconv, np.float32)

    if "nc" not in _CACHE:
        _CACHE["nc"] = _build_nc()
    nc = _CACHE["nc"]

    wdyn, x38, wct, sel = _host_prep(img_embed, cap0, Wred, bred_a,
                                     Wproj, bproj_a, Wconv)
    capn = cap0 / np.linalg.norm(cap0, axis=1, keepdims=True)   # (Q, D)
    bcb = np.ascontiguousarray(
        np.broadcast_to(bconv_a, (128, D))).astype(np.float32)

    in_maps = []
    for c in range(N_CORES):
        qs = slice(c * QL, (c + 1) * QL)
        # w012[p, cc, tap, q] = wdyn[q_local, 128 cc + p, tap]
        w012 = np.ascontiguousarray(
            wdyn[qs].reshape(QL, 8, 128, 3).transpose(2, 1, 3, 0)
        ).astype(np.float32)
        # capn pairs: partitions 0..63 even caption, 64..127 odd
        cpn = np.empty((128, NPAIR, D), np.float32)
        for P in range(NPAIR):
            cpn[0:64, P] = capn[c * QL + 2 * P]
            cpn[64:128, P] = capn[c * QL + 2 * P + 1]
        in_maps.append({
            "x38": x38, "wct": wct, "sel": sel, "w012": w012,
            "capn": cpn, "bcb": bcb,
        })

    trace = bool(int(os.environ.get("KTRACE", "0")))
    tdir = os.environ.get("KTRACE_DIR") or None
    res = run_bass_kernel_spmd(nc, in_maps, core_ids=list(range(N_CORES)),
                               trace=trace, tmpdir=tdir)
    LAST_EXEC_NS = res.exec_time_ns

    sims = np.empty((B, Q), np.float32)
    for c in range(N_CORES):
        o = np.asarray(res.results[c]["out"], np.float32)   # (128, 6)
        for P in range(NPAIR):
            for S in range(2):
                rows = slice(64 * S, 64 * S + 48)
                dot = o[rows, P]
                s2 = o[rows, NPAIR + P]
                sims[:, c * QL + 2 * P + S] = dot / np.sqrt(s2)
    return sims
